# revision 2
# baseline (speedup 1.0000x reference)
"""Trainium2 Bass kernel for MultiHeadSelfAttentionModule (v3).

Same sharding/host contract as the v2 baseline: core = 2*b + g handles batch b
and head-group g (4 heads); host reduces the two head-group partial outputs
and folds the v-bias/out-bias constant row.

v3 restructures the on-device schedule around one invariant: during the
attention stream the ACT engine executes NOTHING but exp instructions.

  - exp runs on uniform [128,1024] psum tiles (128 instrs, ~1038ns each in
    the cost model: 853ns stream + 185ns access bubble). The score ring =
    ps_s pool with 2 slots (4 banks); fill(k+1) only needs exp(k-1) done, so
    the ring self-sustains (fill 427 + sem < exp 1038) as long as PE reaches
    each fill on time.
  - All 16 LayerNorms run in the prologue: x arrives in 4 batched DMAs,
    bn_stats/aggr/recip on DVE, sqrt on ACT (prologue only), x-hat normalize
    on GPSIMD (bf16 out).
  - x-hat transposes: tiles 0-7 on PE (psum borrowed from the pc/mm pools,
    DVE drains); tiles 8-15 via dma_start_transpose (bf16, DMA engines).
  - All psum drains (K/Q/V projections, ctx transposes, out-projection) are
    DVE tensor ops; the Q bias folds into the Q drain (tensor_scalar_add).
  - Projections run as [128,512] psum groups in the mm/pc scratch banks:
    3 groups before the stream (Q j0/j1 slab0, K j0 slab0 - heads 0/1 cover
    the first 32 tiles), the remaining 21 paced 1-per-tile via a side-work
    queue with deadline-ordered seeding.
  - PV is transposed (stationary = exp-weight chunk, moving = [V_h|1], 65
    wide) accumulating into the 2 pc banks per (h,jj) stream; the ones
    column accumulates the softmax denominator; normalize = DVE reciprocal +
    tensor_scalar_mul into bf16 ctx. PV emission lags exp by several tiles
    (et tiles buffer in a deep pool) so the early projection window and the
    pc-slot WAR at stream boundaries never stall PE in front of a ring fill.
  - ctx transposes run per slab pair (2 heads) once both heads of a slab
    finish a jj block: PE transpose into borrowed pc/mm psum, DVE drain into
    ctxT. Out-projection runs in two waves: jj0 mid-stream (borrowed
    scratch), jj1 at the tail (ring and scratch banks are free by then).

PSUM: ring 4 banks + pc 2 + mm 2 = 8.
"""

import math
import sys

if "/opt/trn_rl_repo" not in sys.path:
    sys.path.insert(0, "/opt/trn_rl_repo")

import numpy as np

import concourse.bass as bass
import concourse.mybir as mybir
import concourse.tile as tile
from concourse.bass_utils import run_bass_kernel_spmd
from concourse.masks import make_identity

B, T, D = 4, 2048, 512
H, DK = 8, 64
HPC = 4
DO = HPC * DK
N_CORES = 8
LN_EPS = 1e-5
F32 = mybir.dt.float32
BF16 = mybir.dt.bfloat16
AF = mybir.ActivationFunctionType
ALU = mybir.AluOpType

N_TT = T // 128     # 16 s-chunks
N_CS = D // 128     # 4 contraction slabs
N_IS = DO // 128    # 2 head slabs
W = 1024            # exp tile width (= jj block width)
N_JJ = T // W       # 2
TCH = W // 128      # 8 t-chunks per jj block
ET_BUFS = 26
PV_START = 14       # first tile index at which PV emission begins
PV_PAUSE = 2        # tiles to hold PV after a stream's normalize


def split_multi_waits(nc: bass.Bass) -> None:
    """Hoist all-but-one sync wait from every instruction onto injected
    single-wait NOPs on the same engine, immediately before the owner."""
    ctr = 0
    for fn in nc.m.functions:
        for bb in fn.blocks:
            insts = bb.instructions
            need = any(
                i.sync_info and i.sync_info.on_wait and len(i.sync_info.on_wait) > 1
                for i in insts
            )
            if not need:
                continue
            new = []
            for inst in insts:
                si = inst.sync_info
                if si and si.on_wait and len(si.on_wait) > 1:
                    waits = list(si.on_wait)
                    for w in waits[:-1]:
                        ctr += 1
                        nop = mybir.InstNoOp(
                            name=f"I-wsplit-{ctr}",
                            engine=inst.engine,
                            sync_info=mybir.SyncInfo(on_wait=[w], on_update=[]),
                        )
                        nc.register_instruction(nop)
                        new.append(nop)
                    si.on_wait = [waits[-1]]
                new.append(inst)
            bb.instructions = new


def build_nc() -> bass.Bass:
    nc = bass.Bass()

    xb = nc.declare_dram_parameter("xb", [T, D], F32, isOutput=False)
    wqT = nc.declare_dram_parameter("wqT", [D, DO], BF16, isOutput=False)
    wkT = nc.declare_dram_parameter("wkT", [D, DO], BF16, isOutput=False)
    wvT = nc.declare_dram_parameter("wvT", [D, DO], BF16, isOutput=False)
    woT = nc.declare_dram_parameter("woT", [DO, D], BF16, isOutput=False)
    qb = nc.declare_dram_parameter("qb", [DO, 1], F32, isOutput=False)
    peT4 = nc.declare_dram_parameter("peT4", [DO, T], BF16, isOutput=False)
    out = nc.declare_dram_parameter("out", [T, D], F32, isOutput=True)

    scale = 1.0 / math.sqrt(DK)

    with tile.TileContext(nc) as tc:
        with (
            tc.tile_pool(name="persist", bufs=1) as persist,
            tc.tile_pool(name="lnst", bufs=8) as lnst,
            tc.tile_pool(name="etp", bufs=ET_BUFS) as etp,
            tc.tile_pool(name="ctxp", bufs=10) as ctxp,
            tc.tile_pool(name="rcpp", bufs=4) as rcpp,
            tc.tile_pool(name="outw", bufs=4) as outw,
            tc.tile_pool(name="ps_s", bufs=2, space="PSUM") as ps_s,
            tc.tile_pool(name="ps_pc", bufs=2, space="PSUM") as ps_pc,
            tc.tile_pool(name="ps_mm", bufs=2, space="PSUM") as ps_mm,
        ):
            # ---- persistent SBUF ----
            x_sb = persist.tile([128, N_TT, D], F32)
            xhat = persist.tile([128, N_TT, D], BF16)
            xhatT = persist.tile([128, N_CS, T], BF16)
            KT = persist.tile([128, N_IS, T], BF16)
            QT = persist.tile([128, N_IS, T], BF16)
            Vsb = persist.tile([128, N_TT, HPC * (DK + 1)], BF16)
            ctxT = persist.tile([128, N_IS, T], BF16)
            peT_sb = persist.tile([128, N_IS, T], BF16)
            wkT_sb = persist.tile([128, N_CS, DO], BF16)
            wqT_sb = persist.tile([128, N_CS, DO], BF16)
            wvT_sb = persist.tile([128, N_CS, DO], BF16)
            woT_sb = persist.tile([128, N_IS, D], BF16)
            qb_sb = persist.tile([128, N_IS, 1], F32)

            # ---- input DMAs (batched; the SP HWDGE queue is serial) ----
            xb_r = xb.rearrange("(n p) d -> p n d", p=128)
            peT_r = peT4.rearrange("(s p) t -> p s t", p=128)
            nc.sync.dma_start(out=x_sb[:, 0:4, :], in_=xb_r[:, 0:4, :])
            nc.sync.dma_start(out=x_sb[:, 4:8, :], in_=xb_r[:, 4:8, :])
            nc.sync.dma_start(out=peT_sb[:, :, 0:512], in_=peT_r[:, :, 0:512])
            nc.sync.dma_start(out=wkT_sb, in_=wkT.rearrange("(s p) i -> p s i", p=128))
            nc.sync.dma_start(out=wqT_sb, in_=wqT.rearrange("(s p) i -> p s i", p=128))
            nc.sync.dma_start(out=qb_sb, in_=qb.rearrange("(s p) o -> p s o", p=128))
            nc.sync.dma_start(out=x_sb[:, 8:12, :], in_=xb_r[:, 8:12, :])
            nc.sync.dma_start(out=x_sb[:, 12:16, :], in_=xb_r[:, 12:16, :])
            nc.sync.dma_start(out=peT_sb[:, :, 512:2048], in_=peT_r[:, :, 512:2048])
            nc.sync.dma_start(out=wvT_sb, in_=wvT.rearrange("(s p) i -> p s i", p=128))
            nc.sync.dma_start(out=woT_sb, in_=woT.rearrange("(s p) o -> p s o", p=128))

            # ---- constants ----
            identb = persist.tile([128, 128], BF16)
            make_identity(nc, identb)
            ones_f32 = persist.tile([128, N_TT, HPC], F32)
            nc.vector.memset(ones_f32, 1.0)
            eps_t = persist.tile([128, 1], F32)
            nc.vector.memset(eps_t, LN_EPS)
            nc.vector.tensor_copy(
                out=Vsb.rearrange("p n (h u) -> p n h u", u=DK + 1)[:, :, :, DK],
                in_=ones_f32,
            )

            def pool_tag(pool):
                return "pc" if pool is ps_pc else "mm"

            # ---- LayerNorm (prologue; sqrt is the only prologue ACT work) --
            ln_mv = {}

            def ln_stats(i):
                stats = lnst.tile([128, 6], F32, tag="bn")
                nc.vector.bn_stats(out=stats, in_=x_sb[:, i, :])
                mv = lnst.tile([128, 2], F32, tag="mv")
                nc.vector.bn_aggr(out=mv, in_=stats)
                ln_mv[i] = mv

            def ln_sqrt_fin(i):
                mv = ln_mv.pop(i)
                std = lnst.tile([128, 1], F32, tag="std")
                nc.scalar.activation(out=std, in_=mv[:, 1:2], func=AF.Sqrt, bias=eps_t)
                rstd = lnst.tile([128, 1], F32, tag="rstd")
                nc.vector.reciprocal(out=rstd, in_=std)
                nc.gpsimd.tensor_scalar(
                    out=xhat[:, i, :],
                    in0=x_sb[:, i, :],
                    scalar1=mv[:, 0:1],
                    scalar2=rstd,
                    op0=ALU.subtract,
                    op1=ALU.mult,
                )

            def ln_tile(i):
                ln_stats(i)
                ln_sqrt_fin(i)

            def transpose_pe(i, pool, on_act=False):
                pt = pool.tile([128, 512], BF16, tag=pool_tag(pool),
                               name=f"ptx_{i}")
                for cb in range(N_CS):
                    nc.tensor.transpose(
                        pt[:, cb * 128 : (cb + 1) * 128],
                        xhat[:, i, cb * 128 : (cb + 1) * 128],
                        identb,
                    )
                src_ = pt.rearrange("p (c t) -> p c t", t=128)
                dst = xhatT[:, :, i * 128 : (i + 1) * 128]
                if on_act:
                    nc.scalar.copy(out=dst, in_=src_)
                else:
                    nc.vector.tensor_copy(out=dst, in_=src_)

            def transpose_dma(i):
                nc.sync.dma_start_transpose(
                    out=xhatT[:, :, i * 128 : (i + 1) * 128],
                    in_=xhat[:, i, :],
                )

            # ---- projection groups ([128,512] psum in mm/pc scratch) ----
            def q_group(j, isl, pool):
                tj = slice(j * 512, (j + 1) * 512)
                pq = pool.tile([128, 512], F32, tag=pool_tag(pool),
                               name=f"pq_{j}_{isl}")
                for cs in range(N_CS):
                    nc.tensor.matmul(
                        pq,
                        wqT_sb[:, cs, isl * 128 : (isl + 1) * 128],
                        xhatT[:, cs, tj],
                        start=(cs == 0),
                        stop=(cs == N_CS - 1),
                    )
                nc.vector.tensor_scalar_add(
                    out=QT[:, isl, tj], in0=pq, scalar1=qb_sb[:, isl, :]
                )

            def k_group(j, isl, pool):
                tj = slice(j * 512, (j + 1) * 512)
                pk = pool.tile([128, 512], F32, tag=pool_tag(pool),
                               name=f"pk_{j}_{isl}")
                for cs in range(N_CS):
                    nc.tensor.matmul(
                        pk,
                        wkT_sb[:, cs, isl * 128 : (isl + 1) * 128],
                        xhatT[:, cs, tj],
                        start=(cs == 0),
                        stop=False,
                    )
                nc.tensor.matmul(
                    pk, identb, peT_sb[:, isl, tj], start=False, stop=True
                )
                nc.vector.tensor_copy(out=KT[:, isl, tj], in_=pk)

            def v_group(j, pair, pool):
                st0 = 4 * j + 2 * pair
                pv_ = pool.tile([128, 512], F32, tag=pool_tag(pool),
                                name=f"pvg_{j}_{pair}")
                for k in range(2):
                    st = st0 + k
                    for cs in range(N_CS):
                        nc.tensor.matmul(
                            pv_[:, k * 256 : (k + 1) * 256],
                            xhatT[:, cs, st * 128 : (st + 1) * 128],
                            wvT_sb[:, cs, :],
                            start=(cs == 0),
                            stop=(cs == N_CS - 1),
                        )
                nc.vector.tensor_copy(
                    out=Vsb.rearrange("p n (h u) -> p n h u", u=DK + 1)[
                        :, st0 : st0 + 2, :, 0:DK
                    ],
                    in_=pv_.rearrange("p (s h u) -> p s h u", s=2, u=DK),
                )

            # ---- attention stream pieces ----
            UNITS = [(h, jj, ss) for jj in range(N_JJ) for h in range(HPC)
                     for ss in range(N_TT)]
            et_tiles = {}
            pc_tiles = {}
            ctx_sb = {}

            def fill_unit(u):
                h, jj, ss = u
                hp = slice((h % 2) * 64, (h % 2) * 64 + 64)
                hi = h // 2
                ps = ps_s.tile([128, W], F32, tag="ps", name=f"ps_{h}_{jj}_{ss}")
                for hf in range(W // 512):
                    t0 = jj * W + hf * 512
                    nc.tensor.matmul(
                        ps[:, hf * 512 : (hf + 1) * 512],
                        KT[hp, hi, ss * 128 : (ss + 1) * 128],
                        QT[hp, hi, t0 : t0 + 512],
                        start=True,
                        stop=True,
                    )
                return ps

            def exp_unit(u, ps):
                h, jj, ss = u
                et = etp.tile([128, W], BF16, tag="exp", name=f"et_{h}_{jj}_{ss}")
                nc.scalar.activation(out=et, in_=ps, func=AF.Exp, scale=scale)
                et_tiles[u] = et

            def pv_unit(u):
                h, jj, ss = u
                if (h, jj) not in pc_tiles:
                    pc0 = ps_pc.tile([128, 4, DK + 1], F32, tag="pc",
                                     name=f"pc0_{h}_{jj}")
                    pc1 = ps_pc.tile([128, 4, DK + 1], F32, tag="pc",
                                     name=f"pc1_{h}_{jj}")
                    pc_tiles[(h, jj)] = (pc0, pc1)
                pcs = pc_tiles[(h, jj)]
                et = et_tiles.pop(u)
                for tcn in range(TCH):
                    nc.tensor.matmul(
                        pcs[tcn // 4][:, tcn % 4, :],
                        et[:, tcn * 128 : (tcn + 1) * 128],
                        Vsb[:, ss, h * (DK + 1) : (h + 1) * (DK + 1)],
                        start=(ss == 0 and tcn % 4 == 0),
                        stop=(ss == N_TT - 1 and tcn % 4 == 3),
                    )

            def normalize_half(h, jj, half, pcs):
                if True:
                    pch = pcs[half]
                    rcp = rcpp.tile([128, 4], F32, tag="rcp")
                    nc.vector.reciprocal(out=rcp, in_=pch[:, :, DK])
                    ctxt = ctxp.tile([128, 4, DK], BF16, tag="ctx",
                                     name=f"ctx_{h}_{jj}_{half}")
                    for q in range(4):
                        nc.vector.tensor_scalar_mul(
                            out=ctxt[:, q, :],
                            in0=pch[:, q, 0:DK],
                            scalar1=rcp[:, q : q + 1],
                        )
                    ctx_sb[(h, jj, half)] = ctxt

            def normalize(h, jj):
                pcs = pc_tiles.pop((h, jj))
                for half in range(2):
                    normalize_half(h, jj, half, pcs)

            def head_ctx_transpose(h, jj, tq, pool):
                # 2 t-chunks (2*tq, 2*tq+1) of ONE head into its 64-row band
                isl, hh = h // 2, h % 2
                for tcn in (2 * tq, 2 * tq + 1):
                    half, q = tcn // 4, tcn % 4
                    ptt = pool.tile([64, 128], BF16, tag=pool_tag(pool),
                                    name=f"pth_{h}_{jj}_{tcn}")
                    nc.tensor.transpose(
                        ptt, ctx_sb[(h, jj, half)][:, q, :], identb)
                    nc.vector.tensor_copy(
                        out=ctxT[hh * 64 : (hh + 1) * 64, isl,
                                 jj * W + tcn * 128 : jj * W + (tcn + 1) * 128],
                        in_=ptt,
                    )

            def ctx_transpose_quarter(isl, jj, tq, pool):
                # 2 t-chunks (tq*2, tq*2+1) of both heads of slab isl
                for tcn in (2 * tq, 2 * tq + 1):
                    half, q = tcn // 4, tcn % 4
                    ptt = pool.tile([128, 128], BF16, tag=pool_tag(pool),
                                    name=f"ptt_{isl}_{jj}_{tcn}")
                    for hh in range(2):
                        h = 2 * isl + hh
                        nc.tensor.transpose(
                            ptt[hh * 64 : (hh + 1) * 64, :],
                            ctx_sb[(h, jj, half)][:, q, :],
                            identb,
                        )
                    nc.vector.tensor_copy(
                        out=ctxT[:, isl,
                                 jj * W + tcn * 128 : jj * W + (tcn + 1) * 128],
                        in_=ptt,
                    )

            def out_pair(jj, tcn0, pool, drain_act=False):
                # two out-proj chunks + one batched DMA
                o_t2 = outw.tile([128, 2, D], F32, tag="o",
                                 name=f"o_{jj}_{tcn0}")
                for z in range(2):
                    tcn = tcn0 + z
                    po = pool.tile([128, 512], F32, tag=pool_tag(pool),
                                   name=f"po_{jj}_{tcn}")
                    tsl = slice(jj * W + tcn * 128, jj * W + (tcn + 1) * 128)
                    for isl in range(N_IS):
                        nc.tensor.matmul(
                            po,
                            ctxT[:, isl, tsl],
                            woT_sb[:, isl, :],
                            start=(isl == 0),
                            stop=(isl == N_IS - 1),
                        )
                    if drain_act and z == 0:
                        nc.scalar.copy(out=o_t2[:, z, :], in_=po)
                    else:
                        nc.vector.tensor_copy(out=o_t2[:, z, :], in_=po)
                t0 = jj * W + tcn0 * 128
                nc.sync.dma_start(
                    out=out[t0 : t0 + 256, :].rearrange("(x p) d -> p x d", p=128),
                    in_=o_t2,
                )

            # ================= emission =================
            for i in range(8):
                ln_tile(i)
            for i in range(4):
                transpose_pe(i, ps_mm, on_act=True)
            q_group(0, 0, ps_mm)
            for i in range(4, 8):
                transpose_pe(i, ps_mm)
            q_group(1, 0, ps_mm)
            k_group(0, 0, ps_mm)
            k_group(1, 0, ps_mm)
            for i in range(8, 16):
                ln_stats(i)

            # side work: early slots may use ps_pc (the pc accumulators are
            # not allocated until PV_START); mid-stream work uses ps_mm only.
            def ln_fin_pair(i):
                ln_sqrt_fin(i)
                ln_sqrt_fin(i + 1)
                transpose_dma(i)
                transpose_dma(i + 1)

            side_sched = {
                0: lambda: (k_group(1, 0, ps_mm), ln_fin_pair(8)),
                1: lambda: ln_fin_pair(10),
                2: lambda: ln_fin_pair(12),
                3: lambda: transp_pair(8),
                4: lambda: ln_fin_pair(14),
                5: lambda: transp_pair(10),
                6: lambda: k_group(2, 0, ps_pc),
                7: lambda: transp_pair(12),
                8: lambda: transp_pair(14),
                9: lambda: k_group(3, 0, ps_mm),
                10: lambda: v_group(0, 0, ps_pc),
                12: lambda: v_group(0, 1, ps_mm),
                14: lambda: v_group(1, 0, ps_mm),
                16: lambda: v_group(1, 1, ps_mm),
                18: lambda: v_group(2, 0, ps_mm),
                20: lambda: v_group(2, 1, ps_mm),
                22: lambda: v_group(3, 0, ps_mm),
                24: lambda: v_group(3, 1, ps_mm),
                25: lambda: q_group(0, 1, ps_mm),
                26: lambda: q_group(1, 1, ps_mm),
                28: lambda: k_group(0, 1, ps_mm),
                30: lambda: k_group(1, 1, ps_mm),
                32: lambda: k_group(2, 1, ps_mm),
                34: lambda: k_group(3, 1, ps_mm),
                38: lambda: q_group(2, 0, ps_mm),
                41: lambda: q_group(3, 0, ps_mm),
                44: lambda: q_group(2, 1, ps_mm),
                47: lambda: q_group(3, 1, ps_mm),
            }

            from collections import deque
            side_q = deque()
            pv_q = deque()
            pv_hold = PV_START

            def push_post_stream(h, jj):
                if h == 2 and jj == 1:
                    # head 2's jj1 transposes run during stream (3,1)
                    for tq in range(4):
                        side_q.append(
                            lambda tq=tq: head_ctx_transpose(2, 1, tq, ps_mm))
                    return
                if h == 3 and jj == 1:
                    return  # handled in the tail
                if h in (1, 3):
                    isl = h // 2
                    for tq in range(4):
                        side_q.append(
                            lambda isl=isl, jj=jj, tq=tq:
                            ctx_transpose_quarter(isl, jj, tq, ps_mm))
                if h == 3 and jj == 0:
                    for tcn0 in (0, 2, 4, 6):
                        side_q.append(
                            lambda tcn0=tcn0: out_pair(0, tcn0, ps_mm))

            n_tiles = len(UNITS)
            for k in range(n_tiles):
                u = UNITS[k]
                ps = fill_unit(u)
                exp_unit(u, ps)
                pv_q.append(u)
                if k in side_sched:
                    side_sched[k]()
                elif side_q:
                    side_q.popleft()()
                if k >= pv_hold:
                    if k >= 104:
                        budget = 4
                    elif k in side_sched:
                        budget = 1
                    else:
                        budget = 3
                    while budget > 0 and pv_q:
                        if pv_q[0][2] not in v_done:
                            break
                        nxt = pv_q.popleft()
                        pv_unit(nxt)
                        budget -= 1
                        if nxt[2] == N_TT - 1:
                            if not (nxt[0] == 3 and nxt[1] == 1):
                                normalize(nxt[0], nxt[1])
                                push_post_stream(nxt[0], nxt[1])
                                if k < 100:
                                    pv_hold = k + PV_PAUSE
                            break

            # drain remaining PVs (normalize for (3,1) is handled below)
            while pv_q:
                nxt = pv_q.popleft()
                pv_unit(nxt)
                if nxt[2] == N_TT - 1 and not (nxt[0] == 3 and nxt[1] == 1):
                    normalize(nxt[0], nxt[1])
                    push_post_stream(nxt[0], nxt[1])
            while side_q:
                side_q.popleft()()

            # final stream (3,1) tail: per-half normalize interleaved with
            # head-3 transposes; out-projection through the freed ring and pc
            # banks with ACT+DVE alternating drains; per-chunk output DMA.
            pcs31 = pc_tiles.pop((3, 1))
            normalize_half(3, 1, 0, pcs31)
            for tq in (0, 1):
                head_ctx_transpose(3, 1, tq, ps_mm)
            normalize_half(3, 1, 1, pcs31)
            for tq in (2, 3):
                head_ctx_transpose(3, 1, tq, ps_mm)
            o_t2 = None
            for tcn in range(TCH):
                pool = ps_s if tcn % 2 == 0 else ps_pc
                po = pool.tile([128, 512], F32,
                               tag="ps" if pool is ps_s else "pc",
                               name=f"po_1_{tcn}")
                tsl = slice(W + tcn * 128, W + (tcn + 1) * 128)
                for isl in range(N_IS):
                    nc.tensor.matmul(po, ctxT[:, isl, tsl], woT_sb[:, isl, :],
                                     start=(isl == 0), stop=(isl == N_IS - 1))
                if tcn % 2 == 0:
                    o_t2 = outw.tile([128, 2, D], F32, tag="o",
                                     name=f"ol_{tcn}")
                if tcn % 2 == 1:
                    nc.vector.tensor_copy(out=o_t2[:, 1, :], in_=po)
                else:
                    nc.scalar.copy(out=o_t2[:, 0, :], in_=po)
                if tcn == 5:
                    # last pairs ship per-chunk on alternating queues so the
                    # issue overheads overlap and chunk 7's DMA is minimal
                    t0 = W + 4 * 128
                    nc.scalar.dma_start(
                        out=out[t0 : t0 + 256, :].rearrange(
                            "(x p) d -> p x d", p=128),
                        in_=o_t2,
                    )
                elif tcn == 6:
                    nc.sync.dma_start(out=out[W + 6 * 128 : W + 7 * 128, :],
                                      in_=o_t2[:, 0, :])
                elif tcn == 7:
                    nc.scalar.dma_start(out=out[W + 7 * 128 : W + 8 * 128, :],
                                        in_=o_t2[:, 1, :])
                elif tcn % 2 == 1:
                    t0 = W + (tcn - 1) * 128
                    eng = nc.sync if tcn == 1 else nc.scalar
                    eng.dma_start(
                        out=out[t0 : t0 + 256, :].rearrange(
                            "(x p) d -> p x d", p=128),
                        in_=o_t2,
                    )

    split_multi_waits(nc)
    return nc


def _rel_pos_encoding_np(length: int, d: int) -> np.ndarray:
    pos = np.arange(length, dtype=np.float32)[:, None]
    div = np.exp(
        np.arange(0, d, 2, dtype=np.float32) * np.float32(-(math.log(10000.0) / d))
    ).astype(np.float32)
    ang = pos * div[None, :]
    return np.stack([np.sin(ang), np.cos(ang)], axis=-1).reshape(length, d)


def make_in_maps(x, ln_g, ln_b, wq, bq, wk, bk, wv, bv, wo, bo):
    bf16 = mybir.dt.np(mybir.dt.bfloat16)
    wq_eff = (wq * ln_g[None, :]).astype(np.float32)
    wk_eff = (wk * ln_g[None, :]).astype(np.float32)
    qb_eff = (wq_eff @ ln_b + bq).astype(np.float32)
    wv_eff = (wv * ln_g[None, :]).astype(np.float32)
    pe = _rel_pos_encoding_np(T, DK)
    peT = np.ascontiguousarray(pe.T).astype(bf16)

    in_maps = []
    for c in range(N_CORES):
        b, g = c // 2, c % 2
        hs = slice(g * DO, (g + 1) * DO)
        in_maps.append(
            {
                "xb": np.ascontiguousarray(x[b]).astype(bf16),
                "wqT": np.ascontiguousarray(wq_eff[hs].T).astype(bf16),
                "wkT": np.ascontiguousarray(wk_eff[hs].T).astype(bf16),
                "wvT": np.ascontiguousarray(wv_eff[hs].T).astype(bf16),
                "woT": np.ascontiguousarray(wo[:, hs].T).astype(bf16),
                "qb": np.ascontiguousarray(qb_eff[hs].reshape(DO, 1)),
                "peT": peT,
            }
        )
    return in_maps


def host_combine(results, ln_b, wv, bv, wo, bo):
    vb_eff = wv @ ln_b + bv
    const_row = (vb_eff @ wo.T + bo).astype(np.float32)
    out = np.empty((B, T, D), dtype=np.float32)
    for b in range(B):
        out[b] = (results[2 * b]["out"].astype(np.float32)
                  + results[2 * b + 1]["out"].astype(np.float32) + const_row)
    return out


def kernel(x, ln_g, ln_b, wq, bq, wk, bk, wv, bv, wo, bo, **run_kwargs):
    args = [np.asarray(a, dtype=np.float32) for a in
            (x, ln_g, ln_b, wq, bq, wk, bk, wv, bv, wo, bo)]
    x, ln_g, ln_b, wq, bq, wk, bk, wv, bv, wo, bo = args
    nc = build_nc()
    in_maps = make_in_maps(x, ln_g, ln_b, wq, bq, wk, bk, wv, bv, wo, bo)
    res = run_bass_kernel_spmd(nc, in_maps, core_ids=list(range(N_CORES)), **run_kwargs)
    out = host_combine(res.results, ln_b, wv, bv, wo, bo)
    kernel.last_results = res
    return out


# revision 3
# speedup vs baseline: 1.0447x; 1.0447x over previous
"""Trainium2 Bass kernel for MultiHeadSelfAttentionModule (v3).

Same sharding/host contract as the v2 baseline: core = 2*b + g handles batch b
and head-group g (4 heads); host reduces the two head-group partial outputs
and folds the v-bias/out-bias constant row.

v3 restructures the on-device schedule around one invariant: during the
attention stream the ACT engine executes NOTHING but exp instructions.

  - exp runs on uniform [128,1024] psum tiles (128 instrs, ~1038ns each in
    the cost model: 853ns stream + 185ns access bubble). The score ring =
    ps_s pool with 2 slots (4 banks); fill(k+1) only needs exp(k-1) done, so
    the ring self-sustains (fill 427 + sem < exp 1038) as long as PE reaches
    each fill on time.
  - All 16 LayerNorms run in the prologue: x arrives in 4 batched DMAs,
    bn_stats/aggr/recip on DVE, sqrt on ACT (prologue only), x-hat normalize
    on GPSIMD (bf16 out).
  - x-hat transposes: tiles 0-7 on PE (psum borrowed from the pc/mm pools,
    DVE drains); tiles 8-15 via dma_start_transpose (bf16, DMA engines).
  - All psum drains (K/Q/V projections, ctx transposes, out-projection) are
    DVE tensor ops; the Q bias folds into the Q drain (tensor_scalar_add).
  - Projections run as [128,512] psum groups in the mm/pc scratch banks:
    3 groups before the stream (Q j0/j1 slab0, K j0 slab0 - heads 0/1 cover
    the first 32 tiles), the remaining 21 paced 1-per-tile via a side-work
    queue with deadline-ordered seeding.
  - PV is transposed (stationary = exp-weight chunk, moving = [V_h|1], 65
    wide) accumulating into the 2 pc banks per (h,jj) stream; the ones
    column accumulates the softmax denominator; normalize = DVE reciprocal +
    tensor_scalar_mul into bf16 ctx. PV emission lags exp by several tiles
    (et tiles buffer in a deep pool) so the early projection window and the
    pc-slot WAR at stream boundaries never stall PE in front of a ring fill.
  - ctx transposes run per slab pair (2 heads) once both heads of a slab
    finish a jj block: PE transpose into borrowed pc/mm psum, DVE drain into
    ctxT. Out-projection runs in two waves: jj0 mid-stream (borrowed
    scratch), jj1 at the tail (ring and scratch banks are free by then).

PSUM: ring 4 banks + pc 2 + mm 2 = 8.
"""

import math
import sys

if "/opt/trn_rl_repo" not in sys.path:
    sys.path.insert(0, "/opt/trn_rl_repo")

import numpy as np

import concourse.bass as bass
import concourse.mybir as mybir
import concourse.tile as tile
from concourse.bass_utils import run_bass_kernel_spmd
from concourse.masks import make_identity

B, T, D = 4, 2048, 512
H, DK = 8, 64
HPC = 4
DO = HPC * DK
N_CORES = 8
LN_EPS = 1e-5
F32 = mybir.dt.float32
BF16 = mybir.dt.bfloat16
AF = mybir.ActivationFunctionType
ALU = mybir.AluOpType

N_TT = T // 128     # 16 s-chunks
N_CS = D // 128     # 4 contraction slabs
N_IS = DO // 128    # 2 head slabs
W = 1024            # exp tile width (= jj block width)
N_JJ = T // W       # 2
TCH = W // 128      # 8 t-chunks per jj block
ET_BUFS = 26
PV_START = 14       # first tile index at which PV emission begins
PV_PAUSE = 2        # tiles to hold PV after a stream's normalize


def split_multi_waits(nc: bass.Bass) -> None:
    """Hoist all-but-one sync wait from every instruction onto injected
    single-wait NOPs on the same engine, immediately before the owner."""
    ctr = 0
    for fn in nc.m.functions:
        for bb in fn.blocks:
            insts = bb.instructions
            need = any(
                i.sync_info and i.sync_info.on_wait and len(i.sync_info.on_wait) > 1
                for i in insts
            )
            if not need:
                continue
            new = []
            for inst in insts:
                si = inst.sync_info
                if si and si.on_wait and len(si.on_wait) > 1:
                    waits = list(si.on_wait)
                    for w in waits[:-1]:
                        ctr += 1
                        nop = mybir.InstNoOp(
                            name=f"I-wsplit-{ctr}",
                            engine=inst.engine,
                            sync_info=mybir.SyncInfo(on_wait=[w], on_update=[]),
                        )
                        nc.register_instruction(nop)
                        new.append(nop)
                    si.on_wait = [waits[-1]]
                new.append(inst)
            bb.instructions = new


def build_nc() -> bass.Bass:
    nc = bass.Bass()

    xb = nc.declare_dram_parameter("xb", [T, D], F32, isOutput=False)
    wqT = nc.declare_dram_parameter("wqT", [D, DO], BF16, isOutput=False)
    wkT = nc.declare_dram_parameter("wkT", [D, DO], BF16, isOutput=False)
    wvT = nc.declare_dram_parameter("wvT", [D, DO], BF16, isOutput=False)
    woT = nc.declare_dram_parameter("woT", [DO, D], BF16, isOutput=False)
    qb = nc.declare_dram_parameter("qb", [DO, 1], F32, isOutput=False)
    peT4 = nc.declare_dram_parameter("peT4", [DO, T], BF16, isOutput=False)
    out = nc.declare_dram_parameter("out", [T, D], F32, isOutput=True)

    scale = 1.0 / math.sqrt(DK)

    with tile.TileContext(nc) as tc:
        with (
            tc.tile_pool(name="persist", bufs=1) as persist,
            tc.tile_pool(name="lnst", bufs=8) as lnst,
            tc.tile_pool(name="etp", bufs=ET_BUFS) as etp,
            tc.tile_pool(name="ctxp", bufs=10) as ctxp,
            tc.tile_pool(name="rcpp", bufs=4) as rcpp,
            tc.tile_pool(name="outw", bufs=4) as outw,
            tc.tile_pool(name="ps_s", bufs=2, space="PSUM") as ps_s,
            tc.tile_pool(name="ps_pc", bufs=2, space="PSUM") as ps_pc,
            tc.tile_pool(name="ps_mm", bufs=2, space="PSUM") as ps_mm,
        ):
            # ---- persistent SBUF ----
            x_sb = persist.tile([128, N_TT, D], F32)
            xhat = persist.tile([128, N_TT, D], BF16)
            xhatT = persist.tile([128, N_CS, T], BF16)
            KT = persist.tile([128, N_IS, T], BF16)
            QT = persist.tile([128, N_IS, T], BF16)
            Vsb = persist.tile([128, N_TT, HPC * (DK + 1)], BF16)
            ctxT = persist.tile([128, N_IS, T], BF16)
            peT_sb = persist.tile([128, N_IS, T], BF16)
            wkT_sb = persist.tile([128, N_CS, DO], BF16)
            wqT_sb = persist.tile([128, N_CS, DO], BF16)
            wvT_sb = persist.tile([128, N_CS, DO], BF16)
            woT_sb = persist.tile([128, N_IS, D], BF16)
            qb_sb = persist.tile([128, N_IS, 1], F32)

            # ---- input DMAs (batched; the SP HWDGE queue is serial) ----
            xb_r = xb.rearrange("(n p) d -> p n d", p=128)
            peT_r = peT4.rearrange("(s p) t -> p s t", p=128)
            nc.sync.dma_start(out=x_sb[:, 0:4, :], in_=xb_r[:, 0:4, :])
            nc.sync.dma_start(out=x_sb[:, 4:8, :], in_=xb_r[:, 4:8, :])
            nc.sync.dma_start(out=peT_sb[:, :, 0:512], in_=peT_r[:, :, 0:512])
            nc.sync.dma_start(out=wkT_sb, in_=wkT.rearrange("(s p) i -> p s i", p=128))
            nc.sync.dma_start(out=wqT_sb, in_=wqT.rearrange("(s p) i -> p s i", p=128))
            nc.sync.dma_start(out=qb_sb, in_=qb.rearrange("(s p) o -> p s o", p=128))
            nc.sync.dma_start(out=x_sb[:, 8:12, :], in_=xb_r[:, 8:12, :])
            nc.sync.dma_start(out=x_sb[:, 12:16, :], in_=xb_r[:, 12:16, :])
            nc.sync.dma_start(out=peT_sb[:, :, 512:2048], in_=peT_r[:, :, 512:2048])
            nc.sync.dma_start(out=wvT_sb, in_=wvT.rearrange("(s p) i -> p s i", p=128))
            nc.sync.dma_start(out=woT_sb, in_=woT.rearrange("(s p) o -> p s o", p=128))

            # ---- constants ----
            identb = persist.tile([128, 128], BF16)
            make_identity(nc, identb)
            ones_f32 = persist.tile([128, N_TT, HPC], F32)
            nc.vector.memset(ones_f32, 1.0)
            eps_t = persist.tile([128, 1], F32)
            nc.vector.memset(eps_t, LN_EPS)
            nc.vector.tensor_copy(
                out=Vsb.rearrange("p n (h u) -> p n h u", u=DK + 1)[:, :, :, DK],
                in_=ones_f32,
            )

            def pool_tag(pool):
                return "pc" if pool is ps_pc else "mm"

            # ---- LayerNorm (prologue; sqrt is the only prologue ACT work) --
            ln_mv = {}

            def ln_stats(i):
                stats = lnst.tile([128, 6], F32, tag="bn")
                nc.vector.bn_stats(out=stats, in_=x_sb[:, i, :])
                mv = lnst.tile([128, 2], F32, tag="mv")
                nc.vector.bn_aggr(out=mv, in_=stats)
                ln_mv[i] = mv

            def ln_sqrt_fin(i):
                mv = ln_mv.pop(i)
                std = lnst.tile([128, 1], F32, tag="std")
                nc.scalar.activation(out=std, in_=mv[:, 1:2], func=AF.Sqrt, bias=eps_t)
                rstd = lnst.tile([128, 1], F32, tag="rstd")
                nc.vector.reciprocal(out=rstd, in_=std)
                nc.gpsimd.tensor_scalar(
                    out=xhat[:, i, :],
                    in0=x_sb[:, i, :],
                    scalar1=mv[:, 0:1],
                    scalar2=rstd,
                    op0=ALU.subtract,
                    op1=ALU.mult,
                )

            def ln_tile(i):
                ln_stats(i)
                ln_sqrt_fin(i)

            def transpose_pe(i, pool, on_act=False):
                pt = pool.tile([128, 512], BF16, tag=pool_tag(pool),
                               name=f"ptx_{i}")
                for cb in range(N_CS):
                    nc.tensor.transpose(
                        pt[:, cb * 128 : (cb + 1) * 128],
                        xhat[:, i, cb * 128 : (cb + 1) * 128],
                        identb,
                    )
                src_ = pt.rearrange("p (c t) -> p c t", t=128)
                dst = xhatT[:, :, i * 128 : (i + 1) * 128]
                if on_act:
                    nc.scalar.copy(out=dst, in_=src_)
                else:
                    nc.vector.tensor_copy(out=dst, in_=src_)

            def transpose_dma(i):
                nc.sync.dma_start_transpose(
                    out=xhatT[:, :, i * 128 : (i + 1) * 128],
                    in_=xhat[:, i, :],
                )

            # ---- projection groups ([128,512] psum in mm/pc scratch) ----
            def q_group(j, isl, pool):
                tj = slice(j * 512, (j + 1) * 512)
                pq = pool.tile([128, 512], F32, tag=pool_tag(pool),
                               name=f"pq_{j}_{isl}")
                for cs in range(N_CS):
                    nc.tensor.matmul(
                        pq,
                        wqT_sb[:, cs, isl * 128 : (isl + 1) * 128],
                        xhatT[:, cs, tj],
                        start=(cs == 0),
                        stop=(cs == N_CS - 1),
                    )
                nc.vector.tensor_scalar_add(
                    out=QT[:, isl, tj], in0=pq, scalar1=qb_sb[:, isl, :]
                )

            def k_group(j, isl, pool):
                tj = slice(j * 512, (j + 1) * 512)
                pk = pool.tile([128, 512], F32, tag=pool_tag(pool),
                               name=f"pk_{j}_{isl}")
                for cs in range(N_CS):
                    nc.tensor.matmul(
                        pk,
                        wkT_sb[:, cs, isl * 128 : (isl + 1) * 128],
                        xhatT[:, cs, tj],
                        start=(cs == 0),
                        stop=False,
                    )
                nc.tensor.matmul(
                    pk, identb, peT_sb[:, isl, tj], start=False, stop=True
                )
                nc.vector.tensor_copy(out=KT[:, isl, tj], in_=pk)

            def v_group(j, pair, pool):
                st0 = 4 * j + 2 * pair
                pv_ = pool.tile([128, 512], F32, tag=pool_tag(pool),
                                name=f"pvg_{j}_{pair}")
                for k in range(2):
                    st = st0 + k
                    for cs in range(N_CS):
                        nc.tensor.matmul(
                            pv_[:, k * 256 : (k + 1) * 256],
                            xhatT[:, cs, st * 128 : (st + 1) * 128],
                            wvT_sb[:, cs, :],
                            start=(cs == 0),
                            stop=(cs == N_CS - 1),
                        )
                nc.vector.tensor_copy(
                    out=Vsb.rearrange("p n (h u) -> p n h u", u=DK + 1)[
                        :, st0 : st0 + 2, :, 0:DK
                    ],
                    in_=pv_.rearrange("p (s h u) -> p s h u", s=2, u=DK),
                )

            # ---- attention stream pieces ----
            UNITS = [(h, jj, ss) for jj in range(N_JJ) for h in range(HPC)
                     for ss in range(N_TT)]
            et_tiles = {}
            pc_tiles = {}
            ctx_sb = {}

            def fill_unit(u):
                h, jj, ss = u
                hp = slice((h % 2) * 64, (h % 2) * 64 + 64)
                hi = h // 2
                ps = ps_s.tile([128, W], F32, tag="ps", name=f"ps_{h}_{jj}_{ss}")
                for hf in range(W // 512):
                    t0 = jj * W + hf * 512
                    nc.tensor.matmul(
                        ps[:, hf * 512 : (hf + 1) * 512],
                        KT[hp, hi, ss * 128 : (ss + 1) * 128],
                        QT[hp, hi, t0 : t0 + 512],
                        start=True,
                        stop=True,
                    )
                tc.cur_priority += 10**6
                return ps

            def exp_unit(u, ps):
                h, jj, ss = u
                et = etp.tile([128, W], BF16, tag="exp", name=f"et_{h}_{jj}_{ss}")
                nc.scalar.activation(out=et, in_=ps, func=AF.Exp, scale=scale)
                et_tiles[u] = et

            def pv_unit(u):
                h, jj, ss = u
                if (h, jj) not in pc_tiles:
                    pc0 = ps_pc.tile([128, 4, DK + 1], F32, tag="pc",
                                     name=f"pc0_{h}_{jj}")
                    pc1 = ps_pc.tile([128, 4, DK + 1], F32, tag="pc",
                                     name=f"pc1_{h}_{jj}")
                    pc_tiles[(h, jj)] = (pc0, pc1)
                pcs = pc_tiles[(h, jj)]
                et = et_tiles.pop(u)
                for tcn in range(TCH):
                    nc.tensor.matmul(
                        pcs[tcn // 4][:, tcn % 4, :],
                        et[:, tcn * 128 : (tcn + 1) * 128],
                        Vsb[:, ss, h * (DK + 1) : (h + 1) * (DK + 1)],
                        start=(ss == 0 and tcn % 4 == 0),
                        stop=(ss == N_TT - 1 and tcn % 4 == 3),
                    )

            def normalize_half(h, jj, half, pcs):
                if True:
                    pch = pcs[half]
                    rcp = rcpp.tile([128, 4], F32, tag="rcp")
                    nc.vector.reciprocal(out=rcp, in_=pch[:, :, DK])
                    ctxt = ctxp.tile([128, 4, DK], BF16, tag="ctx",
                                     name=f"ctx_{h}_{jj}_{half}")
                    for q in range(4):
                        nc.vector.tensor_scalar_mul(
                            out=ctxt[:, q, :],
                            in0=pch[:, q, 0:DK],
                            scalar1=rcp[:, q : q + 1],
                        )
                    ctx_sb[(h, jj, half)] = ctxt

            def normalize(h, jj):
                pcs = pc_tiles.pop((h, jj))
                for half in range(2):
                    normalize_half(h, jj, half, pcs)

            def head_ctx_transpose(h, jj, tq, pool):
                # 2 t-chunks (2*tq, 2*tq+1) of ONE head into its 64-row band
                isl, hh = h // 2, h % 2
                for tcn in (2 * tq, 2 * tq + 1):
                    half, q = tcn // 4, tcn % 4
                    ptt = pool.tile([64, 128], BF16, tag=pool_tag(pool),
                                    name=f"pth_{h}_{jj}_{tcn}")
                    nc.tensor.transpose(
                        ptt, ctx_sb[(h, jj, half)][:, q, :], identb)
                    nc.vector.tensor_copy(
                        out=ctxT[hh * 64 : (hh + 1) * 64, isl,
                                 jj * W + tcn * 128 : jj * W + (tcn + 1) * 128],
                        in_=ptt,
                    )

            def ctx_transpose_quarter(isl, jj, tq, pool):
                # 2 t-chunks (tq*2, tq*2+1) of both heads of slab isl
                for tcn in (2 * tq, 2 * tq + 1):
                    half, q = tcn // 4, tcn % 4
                    ptt = pool.tile([128, 128], BF16, tag=pool_tag(pool),
                                    name=f"ptt_{isl}_{jj}_{tcn}")
                    for hh in range(2):
                        h = 2 * isl + hh
                        nc.tensor.transpose(
                            ptt[hh * 64 : (hh + 1) * 64, :],
                            ctx_sb[(h, jj, half)][:, q, :],
                            identb,
                        )
                    nc.vector.tensor_copy(
                        out=ctxT[:, isl,
                                 jj * W + tcn * 128 : jj * W + (tcn + 1) * 128],
                        in_=ptt,
                    )

            def out_pair(jj, tcn0, pool, drain_act=False):
                # two out-proj chunks + one batched DMA
                o_t2 = outw.tile([128, 2, D], F32, tag="o",
                                 name=f"o_{jj}_{tcn0}")
                for z in range(2):
                    tcn = tcn0 + z
                    po = pool.tile([128, 512], F32, tag=pool_tag(pool),
                                   name=f"po_{jj}_{tcn}")
                    tsl = slice(jj * W + tcn * 128, jj * W + (tcn + 1) * 128)
                    for isl in range(N_IS):
                        nc.tensor.matmul(
                            po,
                            ctxT[:, isl, tsl],
                            woT_sb[:, isl, :],
                            start=(isl == 0),
                            stop=(isl == N_IS - 1),
                        )
                    if drain_act and z == 0:
                        nc.scalar.copy(out=o_t2[:, z, :], in_=po)
                    else:
                        nc.vector.tensor_copy(out=o_t2[:, z, :], in_=po)
                t0 = jj * W + tcn0 * 128
                nc.sync.dma_start(
                    out=out[t0 : t0 + 256, :].rearrange("(x p) d -> p x d", p=128),
                    in_=o_t2,
                )

            # ================= emission =================
            for i in range(8):
                ln_tile(i)
            for i in range(4):
                transpose_pe(i, ps_mm, on_act=True)
            q_group(0, 0, ps_mm)
            for i in range(4, 8):
                transpose_pe(i, ps_mm)
            q_group(1, 0, ps_mm)
            k_group(0, 0, ps_mm)
            k_group(1, 0, ps_mm)
            for i in range(8, 16):
                ln_stats(i)

            # side work: early slots may use ps_pc (the pc accumulators are
            # not allocated until PV_START); mid-stream work uses ps_mm only.
            def ln_fin_pair(i):
                ln_sqrt_fin(i)
                ln_sqrt_fin(i + 1)
                transpose_dma(i)
                transpose_dma(i + 1)

            side_sched = {
                0: lambda: (k_group(1, 0, ps_mm), ln_fin_pair(8)),
                1: lambda: ln_fin_pair(10),
                2: lambda: ln_fin_pair(12),
                3: lambda: transp_pair(8),
                4: lambda: ln_fin_pair(14),
                5: lambda: transp_pair(10),
                6: lambda: k_group(2, 0, ps_pc),
                7: lambda: transp_pair(12),
                8: lambda: transp_pair(14),
                9: lambda: k_group(3, 0, ps_mm),
                10: lambda: v_group(0, 0, ps_pc),
                12: lambda: v_group(0, 1, ps_mm),
                14: lambda: v_group(1, 0, ps_mm),
                16: lambda: v_group(1, 1, ps_mm),
                18: lambda: v_group(2, 0, ps_mm),
                20: lambda: v_group(2, 1, ps_mm),
                22: lambda: v_group(3, 0, ps_mm),
                24: lambda: v_group(3, 1, ps_mm),
                25: lambda: q_group(0, 1, ps_mm),
                26: lambda: q_group(1, 1, ps_mm),
                28: lambda: k_group(0, 1, ps_mm),
                30: lambda: k_group(1, 1, ps_mm),
                32: lambda: k_group(2, 1, ps_mm),
                34: lambda: k_group(3, 1, ps_mm),
                38: lambda: q_group(2, 0, ps_mm),
                41: lambda: q_group(3, 0, ps_mm),
                44: lambda: q_group(2, 1, ps_mm),
                47: lambda: q_group(3, 1, ps_mm),
            }

            from collections import deque
            side_q = deque()
            pv_q = deque()
            pv_hold = PV_START

            def push_post_stream(h, jj):
                if h == 2 and jj == 1:
                    # head 2's jj1 transposes run during stream (3,1)
                    for tq in range(4):
                        side_q.append(
                            lambda tq=tq: head_ctx_transpose(2, 1, tq, ps_mm))
                    return
                if h == 3 and jj == 1:
                    return  # handled in the tail
                if h in (1, 3):
                    isl = h // 2
                    for tq in range(4):
                        side_q.append(
                            lambda isl=isl, jj=jj, tq=tq:
                            ctx_transpose_quarter(isl, jj, tq, ps_mm))
                if h == 3 and jj == 0:
                    for tcn0 in (0, 2, 4, 6):
                        side_q.append(
                            lambda tcn0=tcn0: out_pair(0, tcn0, ps_mm))

            n_tiles = len(UNITS)
            for k in range(n_tiles):
                u = UNITS[k]
                ps = fill_unit(u)
                exp_unit(u, ps)
                pv_q.append(u)
                if k in side_sched:
                    side_sched[k]()
                elif side_q:
                    side_q.popleft()()
                if k >= pv_hold:
                    if k >= 104:
                        budget = 4
                    elif k in side_sched:
                        budget = 1
                    else:
                        budget = 3
                    while budget > 0 and pv_q:
                        if pv_q[0][2] not in v_done:
                            break
                        nxt = pv_q.popleft()
                        pv_unit(nxt)
                        budget -= 1
                        if nxt[2] == N_TT - 1:
                            if not (nxt[0] == 3 and nxt[1] == 1):
                                normalize(nxt[0], nxt[1])
                                push_post_stream(nxt[0], nxt[1])
                                if k < 100:
                                    pv_hold = k + PV_PAUSE
                            break

            # drain remaining PVs (normalize for (3,1) is handled below)
            while pv_q:
                nxt = pv_q.popleft()
                pv_unit(nxt)
                if nxt[2] == N_TT - 1 and not (nxt[0] == 3 and nxt[1] == 1):
                    normalize(nxt[0], nxt[1])
                    push_post_stream(nxt[0], nxt[1])
            while side_q:
                side_q.popleft()()

            # final stream (3,1) tail: per-half normalize interleaved with
            # head-3 transposes; out-projection through the freed ring and pc
            # banks with ACT+DVE alternating drains; per-chunk output DMA.
            pcs31 = pc_tiles.pop((3, 1))
            normalize_half(3, 1, 0, pcs31)
            for tq in (0, 1):
                head_ctx_transpose(3, 1, tq, ps_mm)
            normalize_half(3, 1, 1, pcs31)
            for tq in (2, 3):
                head_ctx_transpose(3, 1, tq, ps_mm)
            o_t2 = None
            for tcn in range(TCH):
                pool = ps_s if tcn % 2 == 0 else ps_pc
                po = pool.tile([128, 512], F32,
                               tag="ps" if pool is ps_s else "pc",
                               name=f"po_1_{tcn}")
                tsl = slice(W + tcn * 128, W + (tcn + 1) * 128)
                for isl in range(N_IS):
                    nc.tensor.matmul(po, ctxT[:, isl, tsl], woT_sb[:, isl, :],
                                     start=(isl == 0), stop=(isl == N_IS - 1))
                if tcn % 2 == 0:
                    o_t2 = outw.tile([128, 2, D], F32, tag="o",
                                     name=f"ol_{tcn}")
                if tcn % 2 == 1:
                    nc.vector.tensor_copy(out=o_t2[:, 1, :], in_=po)
                else:
                    nc.scalar.copy(out=o_t2[:, 0, :], in_=po)
                if tcn == 5:
                    # last pairs ship per-chunk on alternating queues so the
                    # issue overheads overlap and chunk 7's DMA is minimal
                    t0 = W + 4 * 128
                    nc.scalar.dma_start(
                        out=out[t0 : t0 + 256, :].rearrange(
                            "(x p) d -> p x d", p=128),
                        in_=o_t2,
                    )
                elif tcn == 6:
                    nc.sync.dma_start(out=out[W + 6 * 128 : W + 7 * 128, :],
                                      in_=o_t2[:, 0, :])
                elif tcn == 7:
                    nc.scalar.dma_start(out=out[W + 7 * 128 : W + 8 * 128, :],
                                        in_=o_t2[:, 1, :])
                elif tcn % 2 == 1:
                    t0 = W + (tcn - 1) * 128
                    eng = nc.sync if tcn == 1 else nc.scalar
                    eng.dma_start(
                        out=out[t0 : t0 + 256, :].rearrange(
                            "(x p) d -> p x d", p=128),
                        in_=o_t2,
                    )

    split_multi_waits(nc)
    return nc


def _rel_pos_encoding_np(length: int, d: int) -> np.ndarray:
    pos = np.arange(length, dtype=np.float32)[:, None]
    div = np.exp(
        np.arange(0, d, 2, dtype=np.float32) * np.float32(-(math.log(10000.0) / d))
    ).astype(np.float32)
    ang = pos * div[None, :]
    return np.stack([np.sin(ang), np.cos(ang)], axis=-1).reshape(length, d)


def make_in_maps(x, ln_g, ln_b, wq, bq, wk, bk, wv, bv, wo, bo):
    bf16 = mybir.dt.np(mybir.dt.bfloat16)
    wq_eff = (wq * ln_g[None, :]).astype(np.float32)
    wk_eff = (wk * ln_g[None, :]).astype(np.float32)
    qb_eff = (wq_eff @ ln_b + bq).astype(np.float32)
    wv_eff = (wv * ln_g[None, :]).astype(np.float32)
    pe = _rel_pos_encoding_np(T, DK)
    peT = np.ascontiguousarray(pe.T).astype(bf16)

    in_maps = []
    for c in range(N_CORES):
        b, g = c // 2, c % 2
        hs = slice(g * DO, (g + 1) * DO)
        in_maps.append(
            {
                "xb": np.ascontiguousarray(x[b]).astype(bf16),
                "wqT": np.ascontiguousarray(wq_eff[hs].T).astype(bf16),
                "wkT": np.ascontiguousarray(wk_eff[hs].T).astype(bf16),
                "wvT": np.ascontiguousarray(wv_eff[hs].T).astype(bf16),
                "woT": np.ascontiguousarray(wo[:, hs].T).astype(bf16),
                "qb": np.ascontiguousarray(qb_eff[hs].reshape(DO, 1)),
                "peT": peT,
            }
        )
    return in_maps


def host_combine(results, ln_b, wv, bv, wo, bo):
    vb_eff = wv @ ln_b + bv
    const_row = (vb_eff @ wo.T + bo).astype(np.float32)
    out = np.empty((B, T, D), dtype=np.float32)
    for b in range(B):
        out[b] = (results[2 * b]["out"].astype(np.float32)
                  + results[2 * b + 1]["out"].astype(np.float32) + const_row)
    return out


def kernel(x, ln_g, ln_b, wq, bq, wk, bk, wv, bv, wo, bo, **run_kwargs):
    args = [np.asarray(a, dtype=np.float32) for a in
            (x, ln_g, ln_b, wq, bq, wk, bk, wv, bv, wo, bo)]
    x, ln_g, ln_b, wq, bq, wk, bk, wv, bv, wo, bo = args
    nc = build_nc()
    in_maps = make_in_maps(x, ln_g, ln_b, wq, bq, wk, bk, wv, bv, wo, bo)
    res = run_bass_kernel_spmd(nc, in_maps, core_ids=list(range(N_CORES)), **run_kwargs)
    out = host_combine(res.results, ln_b, wv, bv, wo, bo)
    kernel.last_results = res
    return out
